# revision 1
# baseline (speedup 1.0000x reference)
"""Trainium2 Bass kernel for autoregressive GMM log-prob (nn_AutoregressiveGMM).

Data-parallel over batch across 8 NeuronCores. Per core (B_loc=2048):
 - first-layer decomposition: inp@W0 = value-masked part (K=i matmul per step)
   + mask part (per-step per-partition bias, host-precomputed cumulative sums)
   + context part (hoisted out of the scan, computed once)
 - 2 residual blocks per step as float32r matmuls (1 cyc/row on PE)
 - GMM head: per-step weight slice, 4 batch-chunks packed into one PSUM tile
   (32-partition groups), (v - means) folded into the head matmul as a K=1 row
 - PE transpose to batch-major, then exp/ln tail; per-step sums stored and the
   final log/subtract/accumulate done once at the end.
"""

import sys

sys.path.insert(0, "/opt/trn_rl_repo")

import numpy as np

import concourse.bass as bass
import concourse.bacc as bacc
import concourse.mybir as mybir
from concourse import tile
from concourse.bass_utils import run_bass_kernel_spmd

B, D, K, H, R, C = 16384, 64, 10, 256, 2, 512
NCORES = 8
BL = B // NCORES          # 2048 rows per core
NCH = BL // 512           # 4 batch chunks of 512
KT = H // 128             # 2 feature tiles
F32 = mybir.dt.float32
F32R = mybir.dt.float32r
BF16 = mybir.dt.bfloat16
LOG2PI = float(np.log(2.0 * np.pi))


def build_graph():
    nc = bacc.Bacc("TRN2", target_bir_lowering=False, debug=False)

    # ---- DRAM parameters (per-core shards / replicated weights) ----
    ctxT_p = nc.declare_dram_parameter("ctxT", [C, BL], BF16, isOutput=False)
    valT_p = nc.declare_dram_parameter("valT", [D, BL], BF16, isOutput=False)
    w0x_p = nc.declare_dram_parameter("w0x", [D, H], BF16, isOutput=False)
    w0c_p = nc.declare_dram_parameter("w0c", [C, H], BF16, isOutput=False)
    wb1_p = nc.declare_dram_parameter("wb1", [R * H, H], BF16, isOutput=False)
    wb2_p = nc.declare_dram_parameter("wb2", [R * H, H], BF16, isOutput=False)
    whp_p = nc.declare_dram_parameter("whp", [H, D * 32], BF16, isOutput=False)
    # packed small tables
    cumbT_p = nc.declare_dram_parameter("cumbT", [128, KT * D], F32, isOutput=False)
    b1T_p = nc.declare_dram_parameter("b1T", [128, R * KT], F32, isOutput=False)
    bhT_p = nc.declare_dram_parameter("bhT", [128, D], F32, isOutput=False)
    vbm_p = nc.declare_dram_parameter("vbm", [128, 16 * D], F32, isOutput=False)
    eye_p = nc.declare_dram_parameter("eye", [128, 128], BF16, isOutput=False)
    eyef_p = nc.declare_dram_parameter("eyef", [128, 128], BF16, isOutput=False)
    out_p = nc.declare_dram_parameter("out", [128, 16], F32, isOutput=True)

    with tile.TileContext(nc) as tc:
        with (
            tc.tile_pool(name="const", bufs=1) as cpool,
            tc.tile_pool(name="state", bufs=2) as spool,
            tc.tile_pool(name="work", bufs=2) as wpool,
            tc.tile_pool(name="psum", bufs=2, space="PSUM") as ppool,
        ):
            # ---- load constants into SBUF ----
            valT = cpool.tile([D, BL], BF16, tag="valT", name="valT")
            nc.sync.dma_start(valT[:], valT_p[:])
            w0x = cpool.tile([D, H], BF16, tag="w0x", name="w0x")
            nc.sync.dma_start(w0x[:], w0x_p[:])
            w0c = [cpool.tile([128, H], BF16, tag=f"w0c{k}", name=f"w0c{k}") for k in range(4)]
            for k in range(4):
                nc.sync.dma_start(w0c[k][:], w0c_p[128 * k:128 * (k + 1), :])
            wb1 = [[cpool.tile([128, H], BF16, tag=f"wb1{r}{k}", name=f"wb1{r}{k}") for k in range(KT)]
                   for r in range(R)]
            wb2 = [[cpool.tile([128, H], BF16, tag=f"wb2{r}{k}", name=f"wb2{r}{k}") for k in range(KT)]
                   for r in range(R)]
            for r in range(R):
                for k in range(KT):
                    nc.sync.dma_start(wb1[r][k][:],
                                      wb1_p[r * H + 128 * k: r * H + 128 * (k + 1), :])
                    nc.sync.dma_start(wb2[r][k][:],
                                      wb2_p[r * H + 128 * k: r * H + 128 * (k + 1), :])
            whp = [cpool.tile([128, D * 32], BF16, tag=f"whp{k}", name=f"whp{k}") for k in range(KT)]
            for k in range(KT):
                nc.sync.dma_start(whp[k][:], whp_p[128 * k:128 * (k + 1), :])
            cumbT = cpool.tile([128, KT * D], F32, tag="cumbT", name="cumbT")
            nc.sync.dma_start(cumbT[:], cumbT_p[:])
            b1T = cpool.tile([128, R * KT], F32, tag="b1T", name="b1T")
            nc.sync.dma_start(b1T[:], b1T_p[:])
            bhT = cpool.tile([128, D], F32, tag="bhT", name="bhT")
            nc.sync.dma_start(bhT[:], bhT_p[:])
            vbm = cpool.tile([128, 16 * D], F32, tag="vbm", name="vbm")
            nc.sync.dma_start(vbm[:], vbm_p[:])
            eye = cpool.tile([128, 128], BF16, tag="eye", name="eye")
            nc.sync.dma_start(eye[:], eye_p[:])
            eyef = cpool.tile([128, 128], BF16, tag="eyef", name="eyef")
            nc.sync.dma_start(eyef[:], eyef_p[:])

            # per-step sums, laid out (128, 16 groups x 64 steps)
            sumE = cpool.tile([128, 16 * D], F32, tag="sumE", name="sumE")
            sumE0 = cpool.tile([128, 16 * D], F32, tag="sumE0", name="sumE0")

            # const bias columns for activation ops
            c_one = cpool.tile([128, 1], F32, tag="c_one", name="c_one")
            nc.vector.memset(c_one[:], 1.00001)
            c_lhalf = cpool.tile([128, 1], F32, tag="c_lhalf", name="c_lhalf")
            nc.vector.memset(c_lhalf[:], float(np.log(0.5)))

            # ---- one-time: ctxproj[f, b] = (context @ W0c)^T ----
            ctxp = [cpool.tile([128, BL], BF16, tag=f"ctxp{n}", name=f"ctxp{n}") for n in range(KT)]
            with tc.tile_pool(name="ctxload", bufs=1) as ctxpool:
                ctxT = [ctxpool.tile([128, BL], BF16, tag=f"ctxT{k}", name=f"ctxT{k}")
                        for k in range(4)]
                for k in range(4):
                    nc.sync.dma_start(ctxT[k][:], ctxT_p[128 * k:128 * (k + 1), :])
                for n in range(KT):
                    for ch in range(NCH):
                        q = ppool.tile([128, 512], F32, tag="pp", name="qc")
                        for k in range(4):
                            nc.tensor.matmul(
                                q[:], w0c[k][:, 128 * n:128 * (n + 1)],
                                ctxT[k][:, 512 * ch:512 * (ch + 1)],
                                start=(k == 0), stop=(k == 3))
                        nc.scalar.activation(ctxp[n][:, 512 * ch:512 * (ch + 1)], q[:],
                                             mybir.ActivationFunctionType.Identity)

            # ---- the 64-step scan ----
            for i in range(D):
                # h, t: (128, KT*BL), feature tile k in cols [k*BL, (k+1)*BL)
                h = spool.tile([128, KT * BL], BF16, tag="h", name="h")
                t = spool.tile([128, KT * BL], BF16, tag="t", name="t")

                # first layer: h0 = relu(maskmm + ctxproj + cumb_i)
                for n in range(KT):
                    for cp in range(2):
                        hsl = h[:, n * BL + 1024 * cp: n * BL + 1024 * (cp + 1)]
                        csl = ctxp[n][:, 1024 * cp:1024 * (cp + 1)]
                        if i > 0:
                            q = ppool.tile([128, 1024], F32, tag="qm", name="qm", bufs=1)
                            for cc in range(2):
                                ch = 2 * cp + cc
                                nc.tensor.matmul(
                                    q[:, 512 * cc:512 * (cc + 1)],
                                    w0x[0:i, 128 * n:128 * (n + 1)],
                                    valT[0:i, 512 * ch:512 * (ch + 1)],
                                    start=True, stop=True)
                            nc.vector.tensor_tensor(hsl, q[:], csl,
                                                    mybir.AluOpType.add)
                            nc.scalar.activation(
                                hsl, hsl, mybir.ActivationFunctionType.Relu,
                                bias=cumbT[:, n * D + i: n * D + i + 1])
                        else:
                            nc.scalar.activation(
                                hsl, csl, mybir.ActivationFunctionType.Relu,
                                bias=cumbT[:, n * D + i: n * D + i + 1])

                # residual blocks
                for r in range(R):
                    for n in range(KT):
                        for cp in range(2):
                            q = ppool.tile([128, 1024], F32, tag="q", name="q", bufs=2)
                            for cc in range(2):
                                ch = 2 * cp + cc
                                for k in range(KT):
                                    nc.tensor.matmul(
                                        q[:, 512 * cc:512 * (cc + 1)],
                                        wb1[r][k][:, 128 * n:128 * (n + 1)],
                                        h[:, k * BL + 512 * ch: k * BL + 512 * (ch + 1)],
                                        start=(k == 0), stop=(k == KT - 1))
                            tsl = t[:, n * BL + 1024 * cp: n * BL + 1024 * (cp + 1)]
                            bcol = b1T[:, r * KT + n: r * KT + n + 1]
                            if r == 0 and cp == 0:
                                nc.vector.tensor_scalar(
                                    tsl, q[:], bcol, 0.0,
                                    op0=mybir.AluOpType.add,
                                    op1=mybir.AluOpType.max)
                            else:
                                nc.scalar.activation(
                                    tsl, q[:],
                                    mybir.ActivationFunctionType.Relu,
                                    bias=bcol)
                    for n in range(KT):
                        for cp in range(2):
                            q = ppool.tile([128, 1024], F32, tag="q", name="q", bufs=2)
                            for cc in range(2):
                                ch = 2 * cp + cc
                                for k in range(KT):
                                    nc.tensor.matmul(
                                        q[:, 512 * cc:512 * (cc + 1)],
                                        wb2[r][k][:, 128 * n:128 * (n + 1)],
                                        t[:, k * BL + 512 * ch: k * BL + 512 * (ch + 1)],
                                        start=(k == 0), stop=(k == KT - 1))
                            sl = h[:, n * BL + 1024 * cp: n * BL + 1024 * (cp + 1)]
                            nc.vector.tensor_tensor(sl, q[:], sl, mybir.AluOpType.add)

                # head: 4 chunks, each into its own psum tile (partitions 0-31)
                psb = wpool.tile([128, 512], BF16, tag="psb", name="psb", bufs=3)
                for ch in range(NCH):
                    pp = ppool.tile([128, 512], F32, tag="pp", name="pp", bufs=2)
                    dst = pp[0:32, :]
                    for k in range(KT):
                        nc.tensor.matmul(
                            dst, whp[k][:, 32 * i:32 * (i + 1)],
                            h[:, k * BL + 512 * ch: k * BL + 512 * (ch + 1)],
                            start=(k == 0), stop=(k == KT - 1))
                    # +head bias while assembling the packed (4x32, 512) layout
                    nc.scalar.activation(psb[32 * ch:32 * (ch + 1), :], dst,
                                         mybir.ActivationFunctionType.Identity,
                                         bias=bhT[32 * ch:32 * (ch + 1), i:i + 1])

                # PE-transpose to batch-major
                pT = ppool.tile([128, 512], BF16, tag="pp", name="pT")
                for cb in range(4):
                    nc.tensor.transpose(pT[:, 128 * cb:128 * (cb + 1)],
                                        psb[:, 128 * cb:128 * (cb + 1)], eyef[:])

                # tail; pT col = 128*cb + 32*ch + j  (j: 0-9 L, 10-19 D=mean-v, 20-29 S)
                pr = pT[:].rearrange("p (cb ch j) -> p cb ch j", ch=4, j=32)
                A = mybir.ActivationFunctionType
                e0 = wpool.tile([128, 160], F32, tag="e0", name="e0")
                er = lambda tl: tl[:].rearrange("p (g j) -> p g j", j=10)
                nc.scalar.activation(er(e0), pr[:, :, :, 0:10], A.Exp)
                se0 = sumE0[:].rearrange("p (g i) -> p g i", i=D)
                nc.vector.tensor_reduce(se0[:, :, i], er(e0),
                                        axis=mybir.AxisListType.X,
                                        op=mybir.AluOpType.add)
                et = wpool.tile([128, 160], F32, tag="et", name="et")
                nc.scalar.activation(er(et), pr[:, :, :, 20:30], A.Exp)
                st = wpool.tile([128, 160], F32, tag="st", name="st")
                nc.scalar.activation(st[:], et[:], A.Ln, bias=c_one[:])
                lns = wpool.tile([128, 160], F32, tag="lns", name="lns")
                nc.scalar.activation(lns[:], st[:], A.Ln)
                inv2 = wpool.tile([128, 160], F32, tag="inv2", name="inv2")
                nc.scalar.activation(inv2[:], lns[:], A.Exp, scale=-2.0,
                                     bias=c_lhalf[:])
                dt_ = wpool.tile([128, 160], F32, tag="dt_", name="dt_")
                vsl = vbm[:, 16 * i:16 * (i + 1)].rearrange(
                    "p (cb ch) -> p cb ch", ch=4)
                nc.vector.tensor_tensor(er(dt_), pr[:, :, :, 10:20],
                                        vsl.to_broadcast((128, 4, 4, 10)),
                                        mybir.AluOpType.subtract)
                sq = wpool.tile([128, 160], F32, tag="sq", name="sq")
                nc.scalar.activation(sq[:], dt_[:], A.Square)
                w = wpool.tile([128, 160], F32, tag="w", name="w")
                nc.vector.tensor_tensor(w[:], sq[:], inv2[:], mybir.AluOpType.mult)
                u = wpool.tile([128, 160], F32, tag="u", name="u")
                nc.vector.tensor_tensor(er(u), pr[:, :, :, 0:10], lns[:].rearrange(
                    "p (g j) -> p g j", j=10), mybir.AluOpType.subtract)
                tt = wpool.tile([128, 160], F32, tag="tt", name="tt")
                nc.vector.tensor_tensor(tt[:], u[:], w[:], mybir.AluOpType.subtract)
                ee = wpool.tile([128, 160], F32, tag="ee", name="ee")
                nc.scalar.activation(ee[:], tt[:], A.Exp)
                se = sumE[:].rearrange("p (g i) -> p g i", i=D)
                nc.vector.tensor_reduce(se[:, :, i], er(ee),
                                        axis=mybir.AxisListType.X,
                                        op=mybir.AluOpType.add)

            # ---- finalize: acc = sum_i [ln(sumE_i) - ln(sumE0_i)] + const ----
            A = mybir.ActivationFunctionType
            nc.scalar.activation(sumE[:], sumE[:], A.Ln)
            nc.scalar.activation(sumE0[:], sumE0[:], A.Ln)
            nc.vector.tensor_tensor(sumE[:], sumE[:], sumE0[:],
                                    mybir.AluOpType.subtract)
            acc = cpool.tile([128, 16], F32, tag="acc", name="acc")
            nc.vector.tensor_reduce(
                acc[:], sumE[:].rearrange("p (g i) -> p g i", i=D),
                axis=mybir.AxisListType.X, op=mybir.AluOpType.add)
            accf = cpool.tile([128, 16], F32, tag="accf", name="accf")
            nc.vector.tensor_scalar(accf[:], acc[:], -0.5 * LOG2PI * D, None,
                                    op0=mybir.AluOpType.add)
            nc.sync.dma_start(out_p[:], accf[:])

    nc.compile()
    # All ACT funcs used here live in one table set; the insertion pass
    # picks per-func first-match sets and thrashes. Rewrite to the combined
    # set and drop redundant loads.
    from concourse.hw_specs import get_activation_tables
    names = list(get_activation_tables(nc.m.arch).keys())
    combined = names.index("natural_log_exp_and_others")
    for b in nc.main_func.blocks:
        keep, first = [], True
        for ins in b.instructions:
            if isinstance(ins, mybir.InstLoadActFuncSet):
                if first:
                    ins.act_func_set_id = combined
                    keep.append(ins)
                    first = False
            else:
                keep.append(ins)
        b.instructions[:] = keep
    return nc


def prep_inputs(value, context, W0, b0, Wb1, bb1, Wb2, bb2, Wh, bh):
    """Host-side weight/layout prep shared by all cores. Returns in_maps."""
    f = np.float32
    W0 = np.asarray(W0, f)
    W0x = np.ascontiguousarray(W0[:D])                      # (64, 256)
    W0m = W0[D:2 * D]                                       # (64, 256)
    W0c = np.ascontiguousarray(W0[2 * D:])                  # (512, 256)
    cum = np.concatenate([np.zeros((1, H), f),
                          np.cumsum(W0m, 0)[:-1]]).astype(f)
    cumb = np.asarray(b0, f)[None, :] + cum                 # (64, 256), row i
    # cumbT[p, n*D + i] = cumb[i, 128n + p]
    cumbT = np.empty((128, KT * D), f)
    for n in range(KT):
        cumbT[:, n * D:(n + 1) * D] = cumb[:, 128 * n:128 * (n + 1)].T
    # effective residual biases (bb2 deferred into next layer / head)
    b1e = np.stack([np.asarray(bb1[0], f),
                    np.asarray(bb1[1], f) + np.asarray(bb2[0], f) @ np.asarray(Wb1[1], f)])
    b1T = np.empty((128, R * KT), f)
    for r in range(R):
        for n in range(KT):
            b1T[:, r * KT + n] = b1e[r, 128 * n:128 * (n + 1)]
    cv = (np.asarray(bb2[0], f) + np.asarray(bb2[1], f))    # (256,)
    Wh_r = np.asarray(Wh, f).reshape(H, D, 3 * K)
    bh_r = np.asarray(bh, f).reshape(D, 3 * K)
    bh_e = bh_r + np.einsum("h,hik->ik", cv, Wh_r)          # (64, 30)
    bh_p = np.zeros((D, 32), f)
    bh_p[:, :30] = bh_e
    # bhT[32*ch + j, i] = bh_p[i, j], replicated over ch
    bhT = np.zeros((128, D), f)
    for ch in range(4):
        bhT[32 * ch:32 * ch + 32, :] = bh_p.T
    Whp = np.zeros((H, D, 32), f)
    Whp[:, :, :30] = Wh_r
    whp = np.ascontiguousarray(Whp.reshape(H, D * 32))
    eye = np.eye(128, dtype=f)
    wb1 = np.asarray(Wb1, f).reshape(R * H, H)
    wb2 = np.asarray(Wb2, f).reshape(R * H, H)

    import ml_dtypes
    bf = ml_dtypes.bfloat16
    value = np.asarray(value, f)
    context = np.asarray(context, f)
    W0xb, W0cb = W0x.astype(bf), W0c.astype(bf)
    wb1b, wb2b, whpb = wb1.astype(bf), wb2.astype(bf), whp.astype(bf)
    eyeb = eye.astype(bf)
    in_maps = []
    for c in range(NCORES):
        sl = slice(c * BL, (c + 1) * BL)
        in_maps.append({
            "ctxT": np.ascontiguousarray(context[sl].T).astype(bf),
            "valT": np.ascontiguousarray(value[sl].T).astype(bf),
            "w0x": W0xb, "w0c": W0cb, "wb1": wb1b, "wb2": wb2b,
            "whp": whpb, "cumbT": cumbT, "b1T": b1T, "bhT": bhT,
            "vbm": np.ascontiguousarray(
                value[sl].reshape(4, 4, 128, D).transpose(2, 3, 1, 0)
                .reshape(128, D * 16)),
            "eye": eyeb, "eyef": eyeb,
        })
    return in_maps


def unpack_out(res_list):
    """res[c]['out'] is (128, 16) with col g: b = (g%4)*512 + (g//4)*128 + bp."""
    full = np.empty(B, np.float32)
    for c, r in enumerate(res_list):
        o = np.asarray(r["out"])          # (128, 16)
        # o[bp, g] -> shard[b]; g = cb*4 + ch; b = ch*512 + cb*128 + bp
        shard = o.reshape(128, 4, 4).transpose(2, 1, 0).reshape(BL)
        full[c * BL:(c + 1) * BL] = shard
    return full


_NC_CACHE = {}


def kernel(**inputs):
    if "nc" not in _NC_CACHE:
        _NC_CACHE["nc"] = build_graph()
    nc = _NC_CACHE["nc"]
    in_maps = prep_inputs(**inputs)
    res = run_bass_kernel_spmd(nc, in_maps, core_ids=list(range(NCORES)))
    return unpack_out(res.results)


if __name__ == "__main__":
    np.random.seed(0)
    fake = {
        "value": np.random.randn(B, D).astype(np.float32),
        "context": np.random.randn(B, C).astype(np.float32),
        "W0": (np.random.randn(2 * D + C, H) * 0.02).astype(np.float32),
        "b0": np.zeros(H, np.float32),
        "Wb1": (np.random.randn(R, H, H) * 0.02).astype(np.float32),
        "bb1": np.zeros((R, H), np.float32),
        "Wb2": (np.random.randn(R, H, H) * 0.02).astype(np.float32),
        "bb2": np.zeros((R, H), np.float32),
        "Wh": (np.random.randn(H, 3 * K * D) * 0.02).astype(np.float32),
        "bh": np.zeros(3 * K * D, np.float32),
    }
    out = kernel(**fake)
    print("out", out.shape, out[:4])

